# revision 4
# baseline (speedup 1.0000x reference)
"""GAT-style message passing kernel for Trainium2 (8 NeuronCores, data-parallel over batch).

Math (per sample, 2 layers, same weights both layers):
    hidden = x @ W_in + b_in                               # [N, H]
    per layer:
        s_j = hidden @ (W_t @ a_j) + b_t.a_j               # xt = hidden@W_t+b_t only feeds scores,
        s_i = hidden @ (W_t @ a_i) + b_t.a_i               # so W_t folds into two matvecs
        score[i,j] = lrelu(s_i[i] + s_j[j])
        att = softmax_j(score)
        hidden = att @ hidden + hidden

Key restructuring for the hardware:
  exp(lrelu(z) - C_i) = max(e^{z-C_i}, e^{0.01 z - C_i}) (exp monotone, lrelu = max(z, .01z))
  and with C_i = s_i + maxS (maxS = max_j s_j), both branches factor rank-1:
      E[j,i] = max( p[j],  p'[j] * g[i] )
      p  = e^{s_j - maxS}                (<= 1)
      p' = e^{0.01 (s_j - maxS)}         (<= 1)
      g  = e^{min(-0.99 (s_i + maxS), 80)}   (clamp exact: binds only when every z<0,
                                              where the common scale cancels in softmax)
  so the whole N^2 pass is ONE fused DVE tensor_scalar op per tile:
      E = (g_bcast * p') max p         with p, p' per-partition scalars.
  att @ hidden + hidden = (E @ hidden) / D + hidden with D = column sums of E
  (ones-matmul on PE), normalization done per-partition in the natural layout.
"""

import numpy as np
from contextlib import ExitStack

S = 2          # samples per core
N = 2048
Din = 20
H = 128
NCH = 16       # j-chunks of 128
NB = 4         # i-blocks
FB = 512       # i-block width
NUM_LAYERS = 2
N_CORES = 8

_compiled = None


def _build(ctx, tc, aps, ctot):
    import concourse.bass as bass
    from concourse import mybir
    from concourse.masks import make_identity

    nc = tc.nc
    f32 = mybir.dt.float32
    Alu = mybir.AluOpType
    Act = mybir.ActivationFunctionType

    x_ap, win_ap, bin_ap, w2_ap, out_ap = aps

    consts = ctx.enter_context(tc.tile_pool(name="consts", bufs=1))
    big = ctx.enter_context(tc.tile_pool(name="big", bufs=4))        # hT / newhT   [128, 2048]
    natp = ctx.enter_context(tc.tile_pool(name="natp", bufs=4))      # h_nat        [128, 16, 128]
    xtp = ctx.enter_context(tc.tile_pool(name="xtp", bufs=2))        # xT           [20, 2048]
    xin = ctx.enter_context(tc.tile_pool(name="xin", bufs=4))        # x load tiles
    gpool = ctx.enter_context(tc.tile_pool(name="gpool", bufs=3))    # gbc [128, 512]
    epool = ctx.enter_context(tc.tile_pool(name="epool", bufs=6))    # E tiles [128, 512]
    ypool = ctx.enter_context(tc.tile_pool(name="ypool", bufs=10))   # Y sbuf [128, 512]
    small = ctx.enter_context(tc.tile_pool(name="small", bufs=10))   # misc small tiles
    psA = ctx.enter_context(tc.tile_pool(name="psA", bufs=2, space="PSUM"))   # [128,512] bcast
    psY = ctx.enter_context(tc.tile_pool(name="psY", bufs=2, space="PSUM"))   # Y accum banks
    psD = ctx.enter_context(tc.tile_pool(name="psD", bufs=1, space="PSUM"))   # D rows
    psT = ctx.enter_context(tc.tile_pool(name="psT", bufs=2, space="PSUM"))   # transposes

    ident = consts.tile([128, 128], f32)
    make_identity(nc, ident)
    ones_r = consts.tile([1, 128], f32)
    nc.vector.memset(ones_r, 1.0)
    ones_c = consts.tile([128, 1], f32)
    nc.vector.memset(ones_c, 1.0)
    win_sb = consts.tile([Din, H], f32)
    nc.sync.dma_start(out=win_sb, in_=win_ap)
    bin_sb = consts.tile([H, 1], f32)
    nc.sync.dma_start(out=bin_sb, in_=bin_ap)
    w2_sb = consts.tile([H, 2], f32)
    nc.sync.dma_start(out=w2_sb, in_=w2_ap)
    # selmat[k, c, p] = (c == k): lhsT selector so matmul(selmat[:, c, :], rows)
    # broadcasts row c of a [16, 128] tile to all 128 output partitions.
    it1 = consts.tile([NCH, NCH, 128], mybir.dt.int32)
    nc.gpsimd.iota(it1, [[1, NCH], [0, 128]], channel_multiplier=0)
    it2 = consts.tile([NCH, NCH, 128], mybir.dt.int32)
    nc.gpsimd.iota(it2, [[0, NCH], [0, 128]], channel_multiplier=1)
    selmat = consts.tile([NCH, NCH, 128], f32)
    nc.vector.tensor_tensor(out=selmat, in0=it1, in1=it2, op=Alu.is_equal)

    def ts(out, in0, s1, s2, op0, op1=None):
        if op1 is None:
            nc.vector.tensor_scalar(out, in0, s1, None, op0)
        else:
            nc.vector.tensor_scalar(out, in0, s1, s2, op0, op1)

    # ---------------- input stage: x -> xT -> hidden0 (both layouts) ---------
    hT = [None, None]      # [128(h), 2048(i)] per sample
    h_nat = [None, None]   # [128(p), 16(c), 128(h)] per sample
    for s in range(S):
        xT = xtp.tile([Din, N], f32)
        for c in range(NCH):
            xt_c = xin.tile([128, Din], f32)
            nc.sync.dma_start(out=xt_c, in_=x_ap[s, c * 128:(c + 1) * 128, :])
            pst = psT.tile([Din, 128], f32, tag="tp")
            nc.tensor.matmul(pst, lhsT=xt_c, rhs=ident, start=True, stop=True)
            nc.scalar.copy(xT[:, c * 128:(c + 1) * 128], pst)
        hTs = big.tile([H, N], f32, tag="hT")
        for b in range(NB):
            psh = psA.tile([H, FB], f32, tag="bcast")
            nc.tensor.matmul(psh, lhsT=win_sb, rhs=xT[:, b * FB:(b + 1) * FB],
                             start=True, stop=True)
            ts(hTs[:, b * FB:(b + 1) * FB], psh, bin_sb[:, 0:1], None, Alu.add)
        hns = natp.tile([128, NCH, H], f32, tag="hnat")
        for c in range(NCH):
            pst = psT.tile([128, 128], f32, tag="tp")
            nc.tensor.matmul(pst, lhsT=hTs[:, c * 128:(c + 1) * 128], rhs=ident,
                             start=True, stop=True)
            nc.scalar.copy(hns[:, c, :], pst)
        hT[s] = hTs
        h_nat[s] = hns

    # ---------------- layers ------------------------------------------------
    for L in range(NUM_LAYERS):
        for s in range(S):
            hTs, hns = hT[s], h_nat[s]

            # scores in [128(j mod 128), 16(chunk), 2] layout (biasless)
            pss = psT.tile([128, 32], f32, tag="tp")
            for c in range(NCH):
                nc.tensor.matmul(pss[:, 2 * c:2 * c + 2],
                                 lhsT=hTs[:, c * 128:(c + 1) * 128], rhs=w2_sb,
                                 start=True, stop=True)
            s0 = small.tile([128, NCH, 2], f32, tag="s0")
            nc.vector.tensor_copy(s0[:, :, :], pss.rearrange("p (c z) -> p c z", z=2))

            # global max of s_j (biasless)
            m1 = small.tile([128, 1], f32, tag="m1")
            nc.vector.tensor_reduce(m1, s0[:, :, 0], axis=mybir.AxisListType.X, op=Alu.max)
            psm = psT.tile([1, 128], f32, tag="tp")
            nc.tensor.matmul(psm, lhsT=m1, rhs=ident, start=True, stop=True)
            m1r = small.tile([1, 128], f32, tag="m1r")
            nc.scalar.copy(m1r, psm)
            mx = small.tile([1, 1], f32, tag="mx")
            nc.vector.tensor_reduce(mx, m1r, axis=mybir.AxisListType.X, op=Alu.max)
            psmb = psT.tile([128, 1], f32, tag="tp")
            nc.tensor.matmul(psmb, lhsT=ones_r, rhs=mx, start=True, stop=True)
            maxbc = small.tile([128, 1], f32, tag="maxbc")
            nc.vector.tensor_copy(maxbc, psmb)
            negmax = small.tile([128, 1], f32, tag="negmax")
            ts(negmax, maxbc, -1.0, None, Alu.mult)
            negmax001 = small.tile([128, 1], f32, tag="negmax001")
            ts(negmax001, maxbc, -0.01, None, Alu.mult)

            # p = exp(s_j - maxS), p' = exp(0.01 (s_j - maxS))   [128, 16]
            p_sb = small.tile([128, NCH], f32, tag="p_sb")
            nc.scalar.activation(p_sb, s0[:, :, 0], Act.Exp, bias=negmax[:, 0:1], scale=1.0)
            pp_sb = small.tile([128, NCH], f32, tag="pp_sb")
            nc.scalar.activation(pp_sb, s0[:, :, 0], Act.Exp, bias=negmax001[:, 0:1], scale=0.01)

            # u = min(-0.99 (s_i + maxS + ctot), 80)   [128, 16] then -> row layout
            u1 = small.tile([128, NCH], f32, tag="u1")
            ts(u1, s0[:, :, 1], maxbc[:, 0:1], float(ctot), Alu.add, Alu.add)
            u_sb = small.tile([128, NCH], f32, tag="u_sb")
            ts(u_sb, u1, -0.99, 80.0, Alu.mult, Alu.min)
            psu = psT.tile([NCH, 128], f32, tag="tp")
            nc.tensor.matmul(psu, lhsT=u_sb, rhs=ident, start=True, stop=True)
            u_rows = small.tile([NCH, 128], f32, tag="u_rows")
            nc.scalar.copy(u_rows, psu)

            dpart_sb = small.tile([128, NCH], f32, tag="dpart")
            y_sb = []
            for b in range(NB):
                # g broadcast tile for this i-block: exp of broadcast u
                ubc = psA.tile([128, FB], f32, tag="bcast")
                for k in range(4):
                    c = 4 * b + k
                    nc.tensor.matmul(ubc[:, k * 128:(k + 1) * 128], lhsT=selmat[:, c, :],
                                     rhs=u_rows, start=True, stop=True)
                gbc = gpool.tile([128, FB], f32, tag="gbc")
                nc.scalar.activation(gbc, ubc, Act.Exp)

                yps = psY.tile([128, FB], f32, tag="yps")
                dps = psD.tile([1, FB], f32, tag="dps")
                for c in range(NCH):
                    e_t = epool.tile([128, FB], f32, tag="e")
                    ts(e_t, gbc, pp_sb[:, c:c + 1], p_sb[:, c:c + 1], Alu.mult, Alu.max)
                    nc.tensor.matmul(yps, lhsT=hns[:, c, :], rhs=e_t,
                                     start=(c == 0), stop=(c == NCH - 1))
                    nc.tensor.matmul(dps, lhsT=ones_c, rhs=e_t,
                                     start=(c == 0), stop=(c == NCH - 1))
                d_row = small.tile([1, FB], f32, tag="d_row")
                nc.scalar.copy(d_row, dps)
                dtp = psT.tile([128, 4], f32, tag="tp")
                for k in range(4):
                    nc.tensor.matmul(dtp[:, k:k + 1], lhsT=d_row[0:1, k * 128:(k + 1) * 128],
                                     rhs=ident[0:1, 0:1], start=True, stop=True)
                nc.vector.tensor_copy(dpart_sb[:, 4 * b:4 * b + 4], dtp)
                ysb = ypool.tile([128, FB], f32, tag="ysb")
                nc.scalar.copy(ysb, yps)
                y_sb.append(ysb)

            rd = small.tile([128, NCH], f32, tag="rd")
            nc.vector.reciprocal(rd, dpart_sb)

            # natural-layout normalize + residual: newh = Ynat * rd + h_nat
            newh_nat = natp.tile([128, NCH, H], f32, tag="hnat")
            for c in range(NCH):
                pst = psT.tile([128, 128], f32, tag="tp")
                nc.tensor.matmul(pst, lhsT=y_sb[c // 4][:, (c % 4) * 128:(c % 4 + 1) * 128],
                                 rhs=ident, start=True, stop=True)
                nc.vector.scalar_tensor_tensor(newh_nat[:, c, :], pst, rd[:, c:c + 1],
                                               hns[:, c, :], Alu.mult, Alu.add)

            if L == 0:
                newhT = big.tile([H, N], f32, tag="hT")
                for c in range(NCH):
                    pst = psT.tile([128, 128], f32, tag="tp")
                    nc.tensor.matmul(pst, lhsT=newh_nat[:, c, :], rhs=ident,
                                     start=True, stop=True)
                    nc.scalar.copy(newhT[:, c * 128:(c + 1) * 128], pst)
                hT[s] = newhT
                h_nat[s] = newh_nat
            else:
                nc.sync.dma_start(
                    out=out_ap[s].rearrange("(c p) h -> p c h", p=128),
                    in_=newh_nat)


def _get_program():
    global _compiled
    if _compiled is not None:
        return _compiled
    import concourse.tile as tile
    from concourse import mybir
    from concourse.bacc import Bacc

    f32 = mybir.dt.float32
    nc = Bacc("TRN2", target_bir_lowering=False, debug=False)
    x_t = nc.dram_tensor("x", [S, N, Din], f32, kind="ExternalInput")
    win_t = nc.dram_tensor("w_in", [Din, H], f32, kind="ExternalInput")
    bin_t = nc.dram_tensor("b_in", [H, 1], f32, kind="ExternalInput")
    w2_t = nc.dram_tensor("w2", [H, 2], f32, kind="ExternalInput")
    out_t = nc.dram_tensor("out", [S, N, H], f32, kind="ExternalOutput")
    aps = (x_t.ap(), win_t.ap(), bin_t.ap(), w2_t.ap(), out_t.ap())
    return nc, aps


def _host_prep(inputs):
    x = np.ascontiguousarray(np.asarray(inputs["x"], dtype=np.float32))
    W_in = np.ascontiguousarray(np.asarray(inputs["W_in"], dtype=np.float32))
    b_in = np.asarray(inputs["b_in"], dtype=np.float32)
    W_t = np.asarray(inputs["W_t"], dtype=np.float32)
    b_t = np.asarray(inputs["b_t"], dtype=np.float32)
    a = np.asarray(inputs["a"], dtype=np.float32)
    a_j, a_i = a[:H, 0], a[H:, 0]
    wj = (W_t @ a_j).astype(np.float32)
    wi = (W_t @ a_i).astype(np.float32)
    w2 = np.ascontiguousarray(np.stack([wj, wi], axis=1))        # [H, 2]
    ctot = float(np.float32(b_t @ a_j) + np.float32(b_t @ a_i))
    b_in_col = np.ascontiguousarray(b_in.reshape(H, 1))
    return x, W_in, b_in_col, w2, ctot


def build_program(ctot):
    from contextlib import ExitStack
    import concourse.tile as tile
    nc, aps = _get_program()
    with tile.TileContext(nc) as tc, ExitStack() as ctx:
        _build(ctx, tc, aps, ctot)
    nc.compile()
    return nc


def kernel(**inputs) -> np.ndarray:
    from concourse.bass_utils import run_bass_kernel_spmd

    x, W_in, b_in_col, w2, ctot = _host_prep(inputs)
    B = x.shape[0]
    nc = build_program(ctot)
    in_maps = []
    for i in range(N_CORES):
        in_maps.append({
            "x": np.ascontiguousarray(x[i * S:(i + 1) * S]),
            "w_in": W_in,
            "b_in": b_in_col,
            "w2": w2,
        })
    res = run_bass_kernel_spmd(nc, in_maps, list(range(N_CORES)))
    out = np.concatenate([res.results[i]["out"] for i in range(N_CORES)], axis=0)
    assert out.shape == (B, N, H)
    return out


# revision 5
# speedup vs baseline: 2.3808x; 2.3808x over previous
"""GAT-style message passing kernel for Trainium2 (8 NeuronCores, data-parallel over batch).

Math (per sample, 2 layers, same weights both layers):
    hidden = x @ W_in + b_in                               # [N, H]
    per layer:
        s_j = hidden @ (W_t @ a_j) + b_t.a_j               # xt = hidden@W_t+b_t only feeds scores,
        s_i = hidden @ (W_t @ a_i) + b_t.a_i               # so W_t folds into two matvecs
        score[i,j] = lrelu(s_i[i] + s_j[j])
        att = softmax_j(score)
        hidden = att @ hidden + hidden

Key restructuring for the hardware:
  exp(lrelu(z) - C_i) = max(e^{z-C_i}, e^{0.01 z - C_i}) (exp monotone, lrelu = max(z, .01z))
  and with C_i = s_i + maxS (maxS = max_j s_j), both branches factor rank-1:
      E[j,i] = max( p[j],  p'[j] * g[i] )
      p  = e^{s_j - maxS}                (<= 1)
      p' = e^{0.01 (s_j - maxS)}         (<= 1)
      g  = e^{min(-0.99 (s_i + maxS), 80)}   (clamp exact: binds only when every z<0,
                                              where the common scale cancels in softmax)
  so the whole N^2 pass is ONE fused DVE tensor_scalar op per tile:
      E = (g_bcast * p') max p         with p, p' per-partition scalars.
  att @ hidden + hidden = (E @ hidden) / D + hidden with D = column sums of E
  (ones-matmul on PE), normalization done per-partition in the natural layout.
"""

import numpy as np
from contextlib import ExitStack

S = 2          # samples per core
N = 2048
Din = 20
H = 128
NCH = 16       # j-chunks of 128
NB = 4         # i-blocks
FB = 512       # i-block width
NUM_LAYERS = 2
N_CORES = 8

_compiled = None


def _build(ctx, tc, aps, ctot):
    import concourse.bass as bass
    from concourse import mybir
    from concourse.masks import make_identity

    nc = tc.nc
    f32 = mybir.dt.float32
    f16 = mybir.dt.float16
    Alu = mybir.AluOpType
    Act = mybir.ActivationFunctionType

    x_ap, win_ap, bin_ap, w2_ap, out_ap = aps

    consts = ctx.enter_context(tc.tile_pool(name="consts", bufs=1))
    big = ctx.enter_context(tc.tile_pool(name="big", bufs=4))        # hT / newhT   [128, 2048]
    natp = ctx.enter_context(tc.tile_pool(name="natp", bufs=4))      # h_nat        [128, 16, 128]
    natp16 = ctx.enter_context(tc.tile_pool(name="natp16", bufs=4))  # h_nat fp16
    xtp = ctx.enter_context(tc.tile_pool(name="xtp", bufs=2))        # xT           [20, 2048]
    xin = ctx.enter_context(tc.tile_pool(name="xin", bufs=4))        # x load tiles
    gpool = ctx.enter_context(tc.tile_pool(name="gpool", bufs=3))    # gbc [128, 512]
    epool = ctx.enter_context(tc.tile_pool(name="epool", bufs=6))    # E tiles [128, 512]
    ypool = ctx.enter_context(tc.tile_pool(name="ypool", bufs=10))   # Y sbuf [128, 512]
    small = ctx.enter_context(tc.tile_pool(name="small", bufs=10))   # misc small tiles
    psA = ctx.enter_context(tc.tile_pool(name="psA", bufs=2, space="PSUM"))   # [128,512] bcast
    psY = ctx.enter_context(tc.tile_pool(name="psY", bufs=2, space="PSUM"))   # Y accum banks
    psD = ctx.enter_context(tc.tile_pool(name="psD", bufs=1, space="PSUM"))   # D rows
    psT = ctx.enter_context(tc.tile_pool(name="psT", bufs=2, space="PSUM"))   # transposes

    ident = consts.tile([128, 128], f32)
    make_identity(nc, ident)
    ones_r = consts.tile([1, 128], f32)
    nc.vector.memset(ones_r, 1.0)
    ones_c = consts.tile([128, 1], f16)
    nc.vector.memset(ones_c, 1.0)
    win_sb = consts.tile([Din, H], f32)
    nc.sync.dma_start(out=win_sb, in_=win_ap)
    bin_sb = consts.tile([H, 1], f32)
    nc.sync.dma_start(out=bin_sb, in_=bin_ap)
    w2_sb = consts.tile([H, 2], f32)
    nc.sync.dma_start(out=w2_sb, in_=w2_ap)
    # selmat[k, c, p] = (c == k): lhsT selector so matmul(selmat[:, c, :], rows)
    # broadcasts row c of a [16, 128] tile to all 128 output partitions.
    it1 = consts.tile([NCH, NCH, 128], mybir.dt.int32)
    nc.gpsimd.iota(it1, [[1, NCH], [0, 128]], channel_multiplier=0)
    it2 = consts.tile([NCH, NCH, 128], mybir.dt.int32)
    nc.gpsimd.iota(it2, [[0, NCH], [0, 128]], channel_multiplier=1)
    selmat = consts.tile([NCH, NCH, 128], f32)
    nc.vector.tensor_tensor(out=selmat, in0=it1, in1=it2, op=Alu.is_equal)

    def ts(out, in0, s1, s2, op0, op1=None):
        if op1 is None:
            nc.vector.tensor_scalar(out, in0, s1, None, op0)
        else:
            nc.vector.tensor_scalar(out, in0, s1, s2, op0, op1)

    # ---------------- input stage: x -> xT -> hidden0 (both layouts) ---------
    hT = [None, None]       # [128(h), 2048(i)] per sample
    h_nat = [None, None]    # [128(p), 16(c), 128(h)] per sample, f32 (residual)
    h_nat16 = [None, None]  # fp16 copy for the attention matmul
    for s in range(S):
        xT = xtp.tile([Din, N], f32)
        for c in range(NCH):
            xt_c = xin.tile([128, Din], f32)
            nc.sync.dma_start(out=xt_c, in_=x_ap[s, c * 128:(c + 1) * 128, :])
            pst = psT.tile([Din, 128], f32, tag="tp")
            nc.tensor.transpose(pst, xt_c, ident)
            nc.scalar.copy(xT[:, c * 128:(c + 1) * 128], pst)
        hTs = big.tile([H, N], f32, tag="hT")
        for b in range(NB):
            psh = psA.tile([H, FB], f32, tag="bcast")
            nc.tensor.matmul(psh, lhsT=win_sb, rhs=xT[:, b * FB:(b + 1) * FB],
                             start=True, stop=True)
            ts(hTs[:, b * FB:(b + 1) * FB], psh, bin_sb[:, 0:1], None, Alu.add)
        hns = natp.tile([128, NCH, H], f32, tag="hnat")
        hns16 = natp16.tile([128, NCH, H], f16, tag="hnat16")
        for c in range(NCH):
            pst = psT.tile([128, 128], f32, tag="tp")
            nc.tensor.transpose(pst, hTs[:, c * 128:(c + 1) * 128], ident)
            nc.scalar.copy(hns[:, c, :], pst)
            nc.vector.tensor_copy(hns16[:, c, :], pst)
        hT[s] = hTs
        h_nat[s] = hns
        h_nat16[s] = hns16

    # ---------------- layers ------------------------------------------------
    for L in range(NUM_LAYERS):
        for s in range(S):
            hTs, hns, hns16 = hT[s], h_nat[s], h_nat16[s]

            # scores in [128(j mod 128), 16(chunk), 2] layout (biasless)
            pss = psT.tile([128, 32], f32, tag="tp")
            for c in range(NCH):
                nc.tensor.matmul(pss[:, 2 * c:2 * c + 2],
                                 lhsT=hTs[:, c * 128:(c + 1) * 128], rhs=w2_sb,
                                 start=True, stop=True)
            s0 = small.tile([128, NCH, 2], f32, tag="s0")
            nc.vector.tensor_copy(s0[:, :, :], pss.rearrange("p (c z) -> p c z", z=2))

            # global max of s_j (biasless)
            m1 = small.tile([128, 1], f32, tag="m1")
            nc.vector.tensor_reduce(m1, s0[:, :, 0], axis=mybir.AxisListType.X, op=Alu.max)
            psm = psT.tile([1, 128], f32, tag="tp")
            nc.tensor.matmul(psm, lhsT=m1, rhs=ident, start=True, stop=True)
            m1r = small.tile([1, 128], f32, tag="m1r")
            nc.scalar.copy(m1r, psm)
            mx = small.tile([1, 1], f32, tag="mx")
            nc.vector.tensor_reduce(mx, m1r, axis=mybir.AxisListType.X, op=Alu.max)
            psmb = psT.tile([128, 1], f32, tag="tp")
            nc.tensor.matmul(psmb, lhsT=ones_r, rhs=mx, start=True, stop=True)
            maxbc = small.tile([128, 1], f32, tag="maxbc")
            nc.vector.tensor_copy(maxbc, psmb)
            negmax = small.tile([128, 1], f32, tag="negmax")
            ts(negmax, maxbc, -1.0, None, Alu.mult)
            negmax001 = small.tile([128, 1], f32, tag="negmax001")
            ts(negmax001, maxbc, -0.01, None, Alu.mult)

            # p = exp(s_j - maxS), p' = exp(0.01 (s_j - maxS))   [128, 16]
            p_sb = small.tile([128, NCH], f32, tag="p_sb")
            nc.scalar.activation(p_sb, s0[:, :, 0], Act.Exp, bias=negmax[:, 0:1], scale=1.0)
            pp_sb = small.tile([128, NCH], f32, tag="pp_sb")
            nc.scalar.activation(pp_sb, s0[:, :, 0], Act.Exp, bias=negmax001[:, 0:1], scale=0.01)

            # u = min(-0.99 (s_i + maxS + ctot), 80)   [128, 16] then -> row layout
            u1 = small.tile([128, NCH], f32, tag="u1")
            ts(u1, s0[:, :, 1], maxbc[:, 0:1], float(ctot), Alu.add, Alu.add)
            u_sb = small.tile([128, NCH], f32, tag="u_sb")
            ts(u_sb, u1, -0.99, 80.0, Alu.mult, Alu.min)
            psu = psT.tile([NCH, 128], f32, tag="tp")
            nc.tensor.matmul(psu, lhsT=u_sb, rhs=ident, start=True, stop=True)
            u_rows = small.tile([NCH, 128], f32, tag="u_rows")
            nc.scalar.copy(u_rows, psu)

            dpart_sb = small.tile([128, NCH], f32, tag="dpart")
            y_sb = []
            for b in range(NB):
                # g broadcast tile for this i-block: exp of broadcast u
                ubc = psA.tile([128, FB], f32, tag="bcast")
                for k in range(4):
                    c = 4 * b + k
                    nc.tensor.matmul(ubc[:, k * 128:(k + 1) * 128], lhsT=selmat[:, c, :],
                                     rhs=u_rows, start=True, stop=True)
                gbc = gpool.tile([128, FB], f32, tag="gbc")
                nc.scalar.activation(gbc, ubc, Act.Exp)

                yps = psY.tile([128, FB], f32, tag="yps")
                dps = psD.tile([1, FB], f32, tag="dps")
                for c in range(NCH):
                    e_t = epool.tile([128, FB], f16, tag="e")
                    ts(e_t, gbc, pp_sb[:, c:c + 1], p_sb[:, c:c + 1], Alu.mult, Alu.max)
                    nc.tensor.matmul(yps, lhsT=hns16[:, c, :], rhs=e_t,
                                     start=(c == 0), stop=(c == NCH - 1))
                    nc.tensor.matmul(dps, lhsT=ones_c, rhs=e_t,
                                     start=(c == 0), stop=(c == NCH - 1))
                d_row = small.tile([1, FB], f32, tag="d_row")
                nc.scalar.copy(d_row, dps)
                dtp = psT.tile([128, 4], f32, tag="tp")
                for k in range(4):
                    nc.tensor.matmul(dtp[:, k:k + 1], lhsT=d_row[0:1, k * 128:(k + 1) * 128],
                                     rhs=ident[0:1, 0:1], start=True, stop=True)
                nc.vector.tensor_copy(dpart_sb[:, 4 * b:4 * b + 4], dtp)
                ysb = ypool.tile([128, FB], f32, tag="ysb")
                nc.scalar.copy(ysb, yps)
                y_sb.append(ysb)

            rd = small.tile([128, NCH], f32, tag="rd")
            nc.vector.reciprocal(rd, dpart_sb)

            # natural-layout normalize + residual: newh = Ynat * rd + h_nat
            newh_nat = natp.tile([128, NCH, H], f32, tag="hnat")
            for c in range(NCH):
                pst = psT.tile([128, 128], f32, tag="tp")
                nc.tensor.transpose(pst, y_sb[c // 4][:, (c % 4) * 128:(c % 4 + 1) * 128],
                                    ident)
                nc.vector.scalar_tensor_tensor(newh_nat[:, c, :], pst, rd[:, c:c + 1],
                                               hns[:, c, :], Alu.mult, Alu.add)

            if L == 0:
                newhT = big.tile([H, N], f32, tag="hT")
                newh16 = natp16.tile([128, NCH, H], f16, tag="hnat16")
                for c in range(NCH):
                    pst = psT.tile([128, 128], f32, tag="tp")
                    nc.tensor.transpose(pst, newh_nat[:, c, :], ident)
                    nc.scalar.copy(newhT[:, c * 128:(c + 1) * 128], pst)
                    nc.vector.tensor_copy(newh16[:, c, :], newh_nat[:, c, :])
                hT[s] = newhT
                h_nat[s] = newh_nat
                h_nat16[s] = newh16
            else:
                nc.sync.dma_start(
                    out=out_ap[s].rearrange("(c p) h -> p c h", p=128),
                    in_=newh_nat)


def _get_program():
    global _compiled
    if _compiled is not None:
        return _compiled
    import concourse.tile as tile
    from concourse import mybir
    from concourse.bacc import Bacc

    f32 = mybir.dt.float32
    nc = Bacc("TRN2", target_bir_lowering=False, debug=False)
    x_t = nc.dram_tensor("x", [S, N, Din], f32, kind="ExternalInput")
    win_t = nc.dram_tensor("w_in", [Din, H], f32, kind="ExternalInput")
    bin_t = nc.dram_tensor("b_in", [H, 1], f32, kind="ExternalInput")
    w2_t = nc.dram_tensor("w2", [H, 2], f32, kind="ExternalInput")
    out_t = nc.dram_tensor("out", [S, N, H], f32, kind="ExternalOutput")
    aps = (x_t.ap(), win_t.ap(), bin_t.ap(), w2_t.ap(), out_t.ap())
    return nc, aps


def _host_prep(inputs):
    x = np.ascontiguousarray(np.asarray(inputs["x"], dtype=np.float32))
    W_in = np.ascontiguousarray(np.asarray(inputs["W_in"], dtype=np.float32))
    b_in = np.asarray(inputs["b_in"], dtype=np.float32)
    W_t = np.asarray(inputs["W_t"], dtype=np.float32)
    b_t = np.asarray(inputs["b_t"], dtype=np.float32)
    a = np.asarray(inputs["a"], dtype=np.float32)
    a_j, a_i = a[:H, 0], a[H:, 0]
    wj = (W_t @ a_j).astype(np.float32)
    wi = (W_t @ a_i).astype(np.float32)
    w2 = np.ascontiguousarray(np.stack([wj, wi], axis=1))        # [H, 2]
    ctot = float(np.float32(b_t @ a_j) + np.float32(b_t @ a_i))
    b_in_col = np.ascontiguousarray(b_in.reshape(H, 1))
    return x, W_in, b_in_col, w2, ctot


def build_program(ctot):
    from contextlib import ExitStack
    import concourse.tile as tile
    nc, aps = _get_program()
    with tile.TileContext(nc) as tc, ExitStack() as ctx:
        _build(ctx, tc, aps, ctot)
    nc.compile()
    return nc


def kernel(**inputs) -> np.ndarray:
    from concourse.bass_utils import run_bass_kernel_spmd

    x, W_in, b_in_col, w2, ctot = _host_prep(inputs)
    B = x.shape[0]
    nc = build_program(ctot)
    in_maps = []
    for i in range(N_CORES):
        in_maps.append({
            "x": np.ascontiguousarray(x[i * S:(i + 1) * S]),
            "w_in": W_in,
            "b_in": b_in_col,
            "w2": w2,
        })
    res = run_bass_kernel_spmd(nc, in_maps, list(range(N_CORES)))
    out = np.concatenate([res.results[i]["out"] for i in range(N_CORES)], axis=0)
    assert out.shape == (B, N, H)
    return out


# revision 9
# speedup vs baseline: 2.9520x; 1.2399x over previous
"""GAT-style message passing kernel for Trainium2 (8 NeuronCores, data-parallel over batch).

Reference math (per sample, 2 layers, shared weights):
    hidden = x @ W_in + b_in                      # [N, H]
    per layer:
        xt  = hidden @ W_t + b_t
        s_j = xt @ a_j ; s_i = xt @ a_i           # xt only feeds the scores
        att = softmax_j(lrelu(s_i[i] + s_j[j]))
        hidden = att @ hidden + hidden

Restructurings used here:
 1) W_t folding: s = hidden @ (W_t a) + b_t.a  — the NxHxH transform collapses.
 2) Rank-21 factorization: hidden == U @ V with V = [W_in; b_in] constant and
    U0 = [x | 1];  per layer U <- att @ U + U  (attention commutes with V).
    All attention matmuls run on U's 21 columns; V is applied once at the end.
    The ones-column of U doubles per layer (att rows sum to 1), and its output
    row in E.T @ U equals 2^L * D — the softmax denominator comes for free.
 3) exp(lrelu(z)-C_i) = max(e^{z-C_i}, e^{0.01z-C_i}) and with C_i = s_i+maxS
    both branches are rank-1:  E[j,i] = max(p[j], p'[j]*g[i])  with
    p = e^{s_j-maxS}, p' = e^{0.01(s_j-maxS)}, g = e^{min(-0.99(s_i+maxS+c),80)}
    so the whole N^2 pass is ONE fused DVE tensor_scalar per tile, no N^2 exp.
 4) s for the next layer from the same product: s' = rD * (Y_U @ w21) + s.
"""

import numpy as np
from contextlib import ExitStack

S = 2          # samples per core
N = 2048
Din = 20
UD = Din + 1   # U columns: 20 x-features + ones
H = 128
NCH = 16       # j-chunks of 128
NB = 4         # i-blocks
FB = 512       # i-block width
NUM_LAYERS = 2
N_CORES = 8


def _build(ctx, tc, aps, ctot):
    import concourse.bass as bass
    from concourse import mybir
    from concourse.masks import make_identity

    nc = tc.nc
    f32 = mybir.dt.float32
    f16 = mybir.dt.float16
    Alu = mybir.AluOpType
    Act = mybir.ActivationFunctionType

    x_ap, w21_ap, v_ap, out_ap = aps

    consts = ctx.enter_context(tc.tile_pool(name="consts", bufs=1))
    utp = ctx.enter_context(tc.tile_pool(name="utp", bufs=2))        # U0T / YUT rows [UD, N]
    natp = ctx.enter_context(tc.tile_pool(name="natp", bufs=4))      # U_nat f32 [128, 16, UD]
    natp16 = ctx.enter_context(tc.tile_pool(name="natp16", bufs=4))  # U_nat fp16
    ynat = ctx.enter_context(tc.tile_pool(name="ynat", bufs=2))      # Ynat f32 [128, 16, UD]
    xin = ctx.enter_context(tc.tile_pool(name="xin", bufs=4))        # x load tiles
    gpool = ctx.enter_context(tc.tile_pool(name="gpool", bufs=3))    # gbc [128, 512]
    epool = ctx.enter_context(tc.tile_pool(name="epool", bufs=6))    # E tiles [128, 512] f16
    outp = ctx.enter_context(tc.tile_pool(name="outp", bufs=2))      # final hidden [128,16,128]
    small = ctx.enter_context(tc.tile_pool(name="small", bufs=12))
    psA = ctx.enter_context(tc.tile_pool(name="psA", bufs=2, space="PSUM"))  # ubc [128,512]
    psU = ctx.enter_context(tc.tile_pool(name="psU", bufs=2, space="PSUM"))  # YUT [UD,512]
    psT = ctx.enter_context(tc.tile_pool(name="psT", bufs=3, space="PSUM"))  # transposes

    ident = consts.tile([128, 128], f32)
    make_identity(nc, ident)
    ones_r = consts.tile([1, 128], f32)
    nc.vector.memset(ones_r, 1.0)
    w21_sb = consts.tile([UD, 2], f32)
    nc.sync.dma_start(out=w21_sb, in_=w21_ap)
    v_sb = consts.tile([UD, H], f32)
    nc.sync.dma_start(out=v_sb, in_=v_ap)
    # selmat[k, c, p] = (c == k): broadcast row c of a [16,128] tile to all
    # 128 output partitions via one K=16 matmul.
    it1 = consts.tile([NCH, NCH, 128], mybir.dt.int32)
    nc.gpsimd.iota(it1, [[1, NCH], [0, 128]], channel_multiplier=0)
    it2 = consts.tile([NCH, NCH, 128], mybir.dt.int32)
    nc.gpsimd.iota(it2, [[0, NCH], [0, 128]], channel_multiplier=1)
    selmat = consts.tile([NCH, NCH, 128], f32)
    nc.vector.tensor_tensor(out=selmat, in0=it1, in1=it2, op=Alu.is_equal)

    def ts(out, in0, s1, s2, op0, op1=None):
        if op1 is None:
            nc.vector.tensor_scalar(out, in0, s1, None, op0)
        else:
            nc.vector.tensor_scalar(out, in0, s1, s2, op0, op1)

    # ------------- input stage: x -> U0 (natural + T), initial scores -------
    u_nat = [None, None]
    u_nat16 = [None, None]
    s_part = [None, None]   # biasless scores [128, 16, 2]
    for s in range(S):
        un = natp.tile([128, NCH, UD], f32, tag="unat")
        nc.vector.memset(un[:, :, Din:UD], 1.0)
        u0t = utp.tile([UD, N], f32, tag="u0t")
        nc.vector.memset(u0t, 1.0)
        for c in range(NCH):
            xt_c = xin.tile([128, Din], f32)
            nc.sync.dma_start(out=xt_c, in_=x_ap[s, c * 128:(c + 1) * 128, :])
            nc.vector.tensor_copy(un[:, c, 0:Din], xt_c)
            pst = psT.tile([Din, 128], f32, tag="tp")
            nc.tensor.transpose(pst, xt_c, ident)
            nc.scalar.copy(u0t[0:Din, c * 128:(c + 1) * 128], pst)
        un16 = natp16.tile([128, NCH, UD], f16, tag="unat16")
        nc.vector.tensor_copy(un16, un)
        # initial biasless scores s0[j, c, z] = U0[j] . w21[:, z]
        pss = psT.tile([128, 32], f32, tag="tp")
        for c in range(NCH):
            nc.tensor.matmul(pss[:, 2 * c:2 * c + 2], lhsT=u0t[:, c * 128:(c + 1) * 128],
                             rhs=w21_sb, start=True, stop=True)
        s0 = small.tile([128, NCH, 2], f32, tag="s0")
        nc.vector.tensor_copy(s0, pss.rearrange("p (c z) -> p c z", z=2))
        u_nat[s], u_nat16[s], s_part[s] = un, un16, s0

    # ------------- layers ---------------------------------------------------
    for L in range(NUM_LAYERS):
        last = L == NUM_LAYERS - 1
        for s in range(S):
            un, un16, s0 = u_nat[s], u_nat16[s], s_part[s]

            # global max of biasless s_j
            m1 = small.tile([128, 1], f32, tag="m1")
            nc.vector.tensor_reduce(m1, s0[:, :, 0], axis=mybir.AxisListType.X, op=Alu.max)
            psm = psT.tile([1, 128], f32, tag="tp")
            nc.tensor.matmul(psm, lhsT=m1, rhs=ident, start=True, stop=True)
            m1r = small.tile([1, 128], f32, tag="m1r")
            nc.scalar.copy(m1r, psm)
            mx = small.tile([1, 1], f32, tag="mx")
            nc.vector.tensor_reduce(mx, m1r, axis=mybir.AxisListType.X, op=Alu.max)
            psmb = psT.tile([128, 1], f32, tag="tp")
            nc.tensor.matmul(psmb, lhsT=ones_r, rhs=mx, start=True, stop=True)
            maxbc = small.tile([128, 1], f32, tag="maxbc")
            nc.vector.tensor_copy(maxbc, psmb)
            negmax = small.tile([128, 1], f32, tag="negmax")
            ts(negmax, maxbc, -1.0, None, Alu.mult)
            negmax001 = small.tile([128, 1], f32, tag="negmax001")
            ts(negmax001, maxbc, -0.01, None, Alu.mult)

            # p = exp(s_j - maxS), p' = exp(0.01(s_j - maxS))
            p_sb = small.tile([128, NCH], f32, tag="p_sb")
            nc.scalar.activation(p_sb, s0[:, :, 0], Act.Exp, bias=negmax[:, 0:1], scale=1.0)
            pp_sb = small.tile([128, NCH], f32, tag="pp_sb")
            nc.scalar.activation(pp_sb, s0[:, :, 0], Act.Exp, bias=negmax001[:, 0:1], scale=0.01)

            # u = min(-0.99(s_i + maxS + ctot), 80), then to row layout
            u1 = small.tile([128, NCH], f32, tag="u1")
            ts(u1, s0[:, :, 1], maxbc[:, 0:1], float(ctot), Alu.add, Alu.add)
            u_sb = small.tile([128, NCH], f32, tag="u_sb")
            ts(u_sb, u1, -0.99, 80.0, Alu.mult, Alu.min)
            psu = psT.tile([NCH, 128], f32, tag="tp")
            nc.tensor.transpose(psu, u_sb, ident)
            u_rows = small.tile([NCH, 128], f32, tag="u_rows")
            nc.scalar.copy(u_rows, psu)

            # attention sweep: Y_UT[u, i] = sum_j U[j, u] E[j, i]
            yut_sb = utp.tile([UD, N], f32, tag="yut")
            for b in range(NB):
                ubc = psA.tile([128, FB], f32, tag="ubc")
                for k in range(4):
                    c = 4 * b + k
                    nc.tensor.matmul(ubc[:, k * 128:(k + 1) * 128], lhsT=selmat[:, c, :],
                                     rhs=u_rows, start=True, stop=True)
                gbc = gpool.tile([128, FB], f32, tag="gbc")
                nc.scalar.activation(gbc, ubc, Act.Exp)

                yps = psU.tile([UD, FB], f32, tag="yps")
                for c in range(NCH):
                    e_t = epool.tile([128, FB], f16, tag="e")
                    ts(e_t, gbc, pp_sb[:, c:c + 1], p_sb[:, c:c + 1], Alu.mult, Alu.max)
                    nc.tensor.matmul(yps, lhsT=un16[:, c, :], rhs=e_t,
                                     start=(c == 0), stop=(c == NCH - 1))
                nc.scalar.copy(yut_sb[:, b * FB:(b + 1) * FB], yps)

            # transpose Y_UT to natural chunks; col Din carries 2^L * D
            yn = ynat.tile([128, NCH, UD], f32, tag="ynat")
            for c in range(NCH):
                pst = psT.tile([128, UD], f32, tag="tp")
                nc.tensor.transpose(pst, yut_sb[:, c * 128:(c + 1) * 128],
                                    ident[0:UD, 0:UD])
                nc.scalar.copy(yn[:, c, :], pst)

            dsc = small.tile([128, NCH], f32, tag="dsc")
            ts(dsc, yn[:, :, Din], float(2.0 ** (-L)), None, Alu.mult)
            rd = small.tile([128, NCH], f32, tag="rd")
            nc.vector.reciprocal(rd, dsc)

            # U' = Ynat * rd + U  (also updates the ones-col to 2^{L+1})
            new_un = natp.tile([128, NCH, UD], f32, tag="unat")
            for c in range(NCH):
                nc.vector.scalar_tensor_tensor(new_un[:, c, :], yn[:, c, :],
                                               rd[:, c:c + 1], un[:, c, :],
                                               Alu.mult, Alu.add)

            if not last:
                new_un16 = natp16.tile([128, NCH, UD], f16, tag="unat16")
                nc.vector.tensor_copy(new_un16, new_un)
                # next-layer biasless scores: s' = rd * (Y_UT @ w21) + s
                psq = psT.tile([128, 32], f32, tag="tp")
                for c in range(NCH):
                    nc.tensor.matmul(psq[:, 2 * c:2 * c + 2],
                                     lhsT=yut_sb[:, c * 128:(c + 1) * 128],
                                     rhs=w21_sb, start=True, stop=True)
                qp = small.tile([128, NCH, 2], f32, tag="qp")
                nc.vector.tensor_copy(qp, psq.rearrange("p (c z) -> p c z", z=2))
                new_s0 = small.tile([128, NCH, 2], f32, tag="s0")
                for c in range(NCH):
                    nc.vector.scalar_tensor_tensor(new_s0[:, c, :], qp[:, c, :],
                                                   rd[:, c:c + 1], s0[:, c, :],
                                                   Alu.mult, Alu.add)
                u_nat[s], u_nat16[s], s_part[s] = new_un, new_un16, new_s0
            else:
                # hidden = U' @ V, emitted per chunk
                hout = outp.tile([128, NCH, H], f32, tag="hout")
                for c in range(NCH):
                    psut = psT.tile([UD, 128], f32, tag="tp")
                    nc.tensor.transpose(psut, new_un[:, c, :], ident)
                    u2t_c = small.tile([UD, 128], f32, tag="u2t")
                    nc.scalar.copy(u2t_c, psut)
                    psh = psT.tile([128, H], f32, tag="tp")
                    nc.tensor.matmul(psh, lhsT=u2t_c, rhs=v_sb, start=True, stop=True)
                    nc.scalar.copy(hout[:, c, :], psh)
                nc.sync.dma_start(
                    out=out_ap[s].rearrange("(c p) h -> p c h", p=128),
                    in_=hout)


def _host_prep(inputs):
    x = np.ascontiguousarray(np.asarray(inputs["x"], dtype=np.float32))
    W_in = np.asarray(inputs["W_in"], dtype=np.float32)
    b_in = np.asarray(inputs["b_in"], dtype=np.float32)
    W_t = np.asarray(inputs["W_t"], dtype=np.float32)
    b_t = np.asarray(inputs["b_t"], dtype=np.float32)
    a = np.asarray(inputs["a"], dtype=np.float32)
    a_j, a_i = a[:H, 0], a[H:, 0]
    wj = (W_t @ a_j).astype(np.float32)
    wi = (W_t @ a_i).astype(np.float32)
    V = np.ascontiguousarray(np.concatenate([W_in, b_in[None, :]], axis=0))  # [21, 128]
    w21 = np.ascontiguousarray(np.stack([V @ wj, V @ wi], axis=1))           # [21, 2]
    ctot = float(np.float32(b_t @ a_j) + np.float32(b_t @ a_i))
    return x, w21, V, ctot


def build_program(ctot):
    import concourse.tile as tile
    from concourse import mybir
    from concourse.bacc import Bacc

    f32 = mybir.dt.float32
    nc = Bacc("TRN2", target_bir_lowering=False, debug=False)
    x_t = nc.dram_tensor("x", [S, N, Din], f32, kind="ExternalInput")
    w21_t = nc.dram_tensor("w21", [UD, 2], f32, kind="ExternalInput")
    v_t = nc.dram_tensor("v", [UD, H], f32, kind="ExternalInput")
    out_t = nc.dram_tensor("out", [S, N, H], f32, kind="ExternalOutput")
    aps = (x_t.ap(), w21_t.ap(), v_t.ap(), out_t.ap())
    with tile.TileContext(nc) as tc, ExitStack() as ctx:
        _build(ctx, tc, aps, ctot)
    nc.compile()
    return nc


def kernel(**inputs) -> np.ndarray:
    from concourse.bass_utils import run_bass_kernel_spmd

    x, w21, V, ctot = _host_prep(inputs)
    B = x.shape[0]
    nc = build_program(ctot)
    in_maps = []
    for i in range(N_CORES):
        in_maps.append({
            "x": np.ascontiguousarray(x[i * S:(i + 1) * S]),
            "w21": w21,
            "v": V,
        })
    res = run_bass_kernel_spmd(nc, in_maps, list(range(N_CORES)))
    out = np.concatenate([res.results[i]["out"] for i in range(N_CORES)], axis=0)
    assert out.shape == (B, N, H)
    return out


# revision 10
# speedup vs baseline: 3.3419x; 1.1321x over previous
"""GAT-style message passing kernel for Trainium2 (8 NeuronCores, data-parallel over batch).

Reference math (per sample, 2 layers, shared weights):
    hidden = x @ W_in + b_in                      # [N, H]
    per layer:
        xt  = hidden @ W_t + b_t
        s_j = xt @ a_j ; s_i = xt @ a_i           # xt only feeds the scores
        att = softmax_j(lrelu(s_i[i] + s_j[j]))
        hidden = att @ hidden + hidden

Restructurings used here:
 1) W_t folding: s = hidden @ (W_t a) + b_t.a  — the NxHxH transform collapses.
 2) Rank-21 factorization: hidden == U @ V with V = [W_in; b_in] constant and
    U0 = [x | 1];  per layer U <- att @ U + U  (attention commutes with V).
    All attention matmuls run on U's 21 columns; V is applied once at the end.
    The ones-column of U doubles per layer (att rows sum to 1), and its output
    row in E.T @ U equals 2^L * D — the softmax denominator comes for free.
 3) exp(lrelu(z)-C_i) = max(e^{z-C_i}, e^{0.01z-C_i}) and with C_i = s_i+maxS
    both branches are rank-1:  E[j,i] = max(p[j], p'[j]*g[i])  with
    p = e^{s_j-maxS}, p' = e^{0.01(s_j-maxS)}, g = e^{min(-0.99(s_i+maxS+c),80)}
    so the whole N^2 pass is ONE fused DVE tensor_scalar per tile, no N^2 exp.
 4) s for the next layer from the same product: s' = rD * (Y_U @ w21) + s.
"""

import numpy as np
from contextlib import ExitStack

S = 2          # samples per core
N = 2048
Din = 20
UD = Din + 1   # U columns: 20 x-features + ones
H = 128
NCH = 16       # j-chunks of 128
NB = 4         # i-blocks
FB = 512       # i-block width
NUM_LAYERS = 2
N_CORES = 8


def _build(ctx, tc, aps, ctot):
    import concourse.bass as bass
    from concourse import mybir
    from concourse.masks import make_identity

    nc = tc.nc
    f32 = mybir.dt.float32
    f16 = mybir.dt.float16
    Alu = mybir.AluOpType
    Act = mybir.ActivationFunctionType

    x_ap, w21_ap, v_ap, out_ap = aps

    consts = ctx.enter_context(tc.tile_pool(name="consts", bufs=1))
    utp = ctx.enter_context(tc.tile_pool(name="utp", bufs=2))        # U0T / YUT rows [UD, N]
    natp = ctx.enter_context(tc.tile_pool(name="natp", bufs=4))      # U_nat f32 [128, 16, UD]
    natp16 = ctx.enter_context(tc.tile_pool(name="natp16", bufs=4))  # U_nat fp16
    ynat = ctx.enter_context(tc.tile_pool(name="ynat", bufs=2))      # Ynat f32 [128, 16, UD]
    xin = ctx.enter_context(tc.tile_pool(name="xin", bufs=4))        # x load tiles
    gpool = ctx.enter_context(tc.tile_pool(name="gpool", bufs=4))    # gbc [128, 512]
    epool = ctx.enter_context(tc.tile_pool(name="epool", bufs=8))    # E tiles [128, 512] f16
    outp = ctx.enter_context(tc.tile_pool(name="outp", bufs=2))      # final hidden [128,16,128]
    small = ctx.enter_context(tc.tile_pool(name="small", bufs=12))
    psA = ctx.enter_context(tc.tile_pool(name="psA", bufs=2, space="PSUM"))  # ubc [128,512]
    psU = ctx.enter_context(tc.tile_pool(name="psU", bufs=3, space="PSUM"))  # YUT [UD,512]
    psT = ctx.enter_context(tc.tile_pool(name="psT", bufs=3, space="PSUM"))  # transposes

    ident = consts.tile([128, 128], f32)
    make_identity(nc, ident)
    ones_r = consts.tile([1, 128], f32)
    nc.vector.memset(ones_r, 1.0)
    w21_sb = consts.tile([UD, 2], f32)
    nc.sync.dma_start(out=w21_sb, in_=w21_ap)
    v_sb = consts.tile([UD, H], f32)
    nc.sync.dma_start(out=v_sb, in_=v_ap)
    # selmat[k, c, p] = (c == k): broadcast row c of a [16,128] tile to all
    # 128 output partitions via one K=16 matmul.
    it1 = consts.tile([NCH, NCH, 128], mybir.dt.int32)
    nc.gpsimd.iota(it1, [[1, NCH], [0, 128]], channel_multiplier=0)
    it2 = consts.tile([NCH, NCH, 128], mybir.dt.int32)
    nc.gpsimd.iota(it2, [[0, NCH], [0, 128]], channel_multiplier=1)
    selmat = consts.tile([NCH, NCH, 128], f32)
    nc.vector.tensor_tensor(out=selmat, in0=it1, in1=it2, op=Alu.is_equal)

    def ts(out, in0, s1, s2, op0, op1=None):
        if op1 is None:
            nc.vector.tensor_scalar(out, in0, s1, None, op0)
        else:
            nc.vector.tensor_scalar(out, in0, s1, s2, op0, op1)

    # ------------- input stage: x -> U0 (natural + T), initial scores -------
    u_nat = [None, None]
    u_nat16 = [None, None]
    s_part = [None, None]   # biasless scores [128, 16, 2]
    for s in range(S):
        un = natp.tile([128, NCH, UD], f32, tag="unat")
        nc.vector.memset(un[:, :, Din:UD], 1.0)
        u0t = utp.tile([UD, N], f32, tag="u0t")
        nc.vector.memset(u0t, 1.0)
        for c in range(NCH):
            xt_c = xin.tile([128, Din], f32)
            nc.sync.dma_start(out=xt_c, in_=x_ap[s, c * 128:(c + 1) * 128, :])
            nc.vector.tensor_copy(un[:, c, 0:Din], xt_c)
            pst = psT.tile([Din, 128], f32, tag="tp")
            nc.tensor.transpose(pst, xt_c, ident)
            nc.scalar.copy(u0t[0:Din, c * 128:(c + 1) * 128], pst)
        un16 = natp16.tile([128, NCH, UD], f16, tag="unat16")
        nc.vector.tensor_copy(un16, un)
        # initial biasless scores s0[j, c, z] = U0[j] . w21[:, z]
        pss = psT.tile([128, 32], f32, tag="tp")
        for c in range(NCH):
            nc.tensor.matmul(pss[:, 2 * c:2 * c + 2], lhsT=u0t[:, c * 128:(c + 1) * 128],
                             rhs=w21_sb, start=True, stop=True)
        s0 = small.tile([128, NCH, 2], f32, tag="s0")
        nc.vector.tensor_copy(s0, pss.rearrange("p (c z) -> p c z", z=2))
        u_nat[s], u_nat16[s], s_part[s] = un, un16, s0

    # ------------- layers ---------------------------------------------------
    for L in range(NUM_LAYERS):
        last = L == NUM_LAYERS - 1
        for s in range(S):
            un, un16, s0 = u_nat[s], u_nat16[s], s_part[s]

            # global max of biasless s_j
            m1 = small.tile([128, 1], f32, tag="m1")
            nc.vector.tensor_reduce(m1, s0[:, :, 0], axis=mybir.AxisListType.X, op=Alu.max)
            psm = psT.tile([1, 128], f32, tag="tp")
            nc.tensor.matmul(psm, lhsT=m1, rhs=ident, start=True, stop=True)
            m1r = small.tile([1, 128], f32, tag="m1r")
            nc.scalar.copy(m1r, psm)
            mx = small.tile([1, 1], f32, tag="mx")
            nc.vector.tensor_reduce(mx, m1r, axis=mybir.AxisListType.X, op=Alu.max)
            psmb = psT.tile([128, 1], f32, tag="tp")
            nc.tensor.matmul(psmb, lhsT=ones_r, rhs=mx, start=True, stop=True)
            maxbc = small.tile([128, 1], f32, tag="maxbc")
            nc.vector.tensor_copy(maxbc, psmb)
            negmax = small.tile([128, 1], f32, tag="negmax")
            ts(negmax, maxbc, -1.0, None, Alu.mult)
            negmax001 = small.tile([128, 1], f32, tag="negmax001")
            ts(negmax001, maxbc, -0.01, None, Alu.mult)

            # p = exp(s_j - maxS), p' = exp(0.01(s_j - maxS))
            p_sb = small.tile([128, NCH], f32, tag="p_sb")
            nc.scalar.activation(p_sb, s0[:, :, 0], Act.Exp, bias=negmax[:, 0:1], scale=1.0)
            pp_sb = small.tile([128, NCH], f32, tag="pp_sb")
            nc.scalar.activation(pp_sb, s0[:, :, 0], Act.Exp, bias=negmax001[:, 0:1], scale=0.01)

            # u = min(-0.99(s_i + maxS + ctot), 80), then to row layout
            u1 = small.tile([128, NCH], f32, tag="u1")
            ts(u1, s0[:, :, 1], maxbc[:, 0:1], float(ctot), Alu.add, Alu.add)
            u_sb = small.tile([128, NCH], f32, tag="u_sb")
            ts(u_sb, u1, -0.99, 10.5, Alu.mult, Alu.min)
            psu = psT.tile([NCH, 128], f32, tag="tp")
            nc.tensor.transpose(psu, u_sb, ident)
            u_rows = small.tile([NCH, 128], f32, tag="u_rows")
            nc.scalar.copy(u_rows, psu)

            # attention sweep: Y_UT[u, i] = sum_j U[j, u] E[j, i]
            yut_sb = utp.tile([UD, N], f32, tag="yut")
            for b in range(NB):
                ubc = psA.tile([128, FB], f32, tag="ubc")
                for k in range(4):
                    c = 4 * b + k
                    nc.tensor.matmul(ubc[:, k * 128:(k + 1) * 128], lhsT=selmat[:, c, :],
                                     rhs=u_rows, start=True, stop=True)
                gbc = gpool.tile([128, FB], f16, tag="gbc")
                nc.scalar.activation(gbc, ubc, Act.Exp)

                yps = psU.tile([UD, FB], f32, tag="yps")
                for c in range(NCH):
                    e_t = epool.tile([128, FB], f16, tag="e")
                    ts(e_t, gbc, pp_sb[:, c:c + 1], p_sb[:, c:c + 1], Alu.mult, Alu.max)
                    nc.tensor.matmul(yps, lhsT=un16[:, c, :], rhs=e_t,
                                     start=(c == 0), stop=(c == NCH - 1))
                nc.scalar.copy(yut_sb[:, b * FB:(b + 1) * FB], yps)

            # transpose Y_UT to natural chunks; col Din carries 2^L * D
            yn = ynat.tile([128, NCH, UD], f32, tag="ynat")
            for c in range(NCH):
                pst = psT.tile([128, UD], f32, tag="tp")
                nc.tensor.transpose(pst, yut_sb[:, c * 128:(c + 1) * 128],
                                    ident[0:UD, 0:UD])
                nc.vector.tensor_copy(yn[:, c, :], pst)

            dsc = small.tile([128, NCH], f32, tag="dsc")
            ts(dsc, yn[:, :, Din], float(2.0 ** (-L)), None, Alu.mult)
            rd = small.tile([128, NCH], f32, tag="rd")
            nc.vector.reciprocal(rd, dsc)

            # U' = Ynat * rd + U  (also updates the ones-col to 2^{L+1})
            new_un = natp.tile([128, NCH, UD], f32, tag="unat")
            for c in range(NCH):
                nc.vector.scalar_tensor_tensor(new_un[:, c, :], yn[:, c, :],
                                               rd[:, c:c + 1], un[:, c, :],
                                               Alu.mult, Alu.add)

            if not last:
                new_un16 = natp16.tile([128, NCH, UD], f16, tag="unat16")
                nc.vector.tensor_copy(new_un16, new_un)
                # next-layer biasless scores: s' = rd * (Y_UT @ w21) + s
                psq = psT.tile([128, 32], f32, tag="tp")
                for c in range(NCH):
                    nc.tensor.matmul(psq[:, 2 * c:2 * c + 2],
                                     lhsT=yut_sb[:, c * 128:(c + 1) * 128],
                                     rhs=w21_sb, start=True, stop=True)
                qp = small.tile([128, NCH, 2], f32, tag="qp")
                nc.vector.tensor_copy(qp, psq.rearrange("p (c z) -> p c z", z=2))
                new_s0 = small.tile([128, NCH, 2], f32, tag="s0")
                for c in range(NCH):
                    nc.vector.scalar_tensor_tensor(new_s0[:, c, :], qp[:, c, :],
                                                   rd[:, c:c + 1], s0[:, c, :],
                                                   Alu.mult, Alu.add)
                u_nat[s], u_nat16[s], s_part[s] = new_un, new_un16, new_s0
            else:
                # hidden = U' @ V, emitted per chunk
                hout = outp.tile([128, NCH, H], f32, tag="hout")
                for c in range(NCH):
                    psut = psT.tile([UD, 128], f32, tag="tp")
                    nc.tensor.transpose(psut, new_un[:, c, :], ident)
                    u2t_c = small.tile([UD, 128], f32, tag="u2t")
                    nc.vector.tensor_copy(u2t_c, psut)
                    psh = psT.tile([128, H], f32, tag="tp")
                    nc.tensor.matmul(psh, lhsT=u2t_c, rhs=v_sb, start=True, stop=True)
                    nc.vector.tensor_copy(hout[:, c, :], psh)
                nc.sync.dma_start(
                    out=out_ap[s].rearrange("(c p) h -> p c h", p=128),
                    in_=hout)


def _host_prep(inputs):
    x = np.ascontiguousarray(np.asarray(inputs["x"], dtype=np.float32))
    W_in = np.asarray(inputs["W_in"], dtype=np.float32)
    b_in = np.asarray(inputs["b_in"], dtype=np.float32)
    W_t = np.asarray(inputs["W_t"], dtype=np.float32)
    b_t = np.asarray(inputs["b_t"], dtype=np.float32)
    a = np.asarray(inputs["a"], dtype=np.float32)
    a_j, a_i = a[:H, 0], a[H:, 0]
    wj = (W_t @ a_j).astype(np.float32)
    wi = (W_t @ a_i).astype(np.float32)
    V = np.ascontiguousarray(np.concatenate([W_in, b_in[None, :]], axis=0))  # [21, 128]
    w21 = np.ascontiguousarray(np.stack([V @ wj, V @ wi], axis=1))           # [21, 2]
    ctot = float(np.float32(b_t @ a_j) + np.float32(b_t @ a_i))
    return x, w21, V, ctot


def build_program(ctot):
    import concourse.tile as tile
    from concourse import mybir
    from concourse.bacc import Bacc

    f32 = mybir.dt.float32
    nc = Bacc("TRN2", target_bir_lowering=False, debug=False)
    x_t = nc.dram_tensor("x", [S, N, Din], f32, kind="ExternalInput")
    w21_t = nc.dram_tensor("w21", [UD, 2], f32, kind="ExternalInput")
    v_t = nc.dram_tensor("v", [UD, H], f32, kind="ExternalInput")
    out_t = nc.dram_tensor("out", [S, N, H], f32, kind="ExternalOutput")
    aps = (x_t.ap(), w21_t.ap(), v_t.ap(), out_t.ap())
    with tile.TileContext(nc) as tc, ExitStack() as ctx:
        _build(ctx, tc, aps, ctot)
    nc.compile()
    return nc


def kernel(**inputs) -> np.ndarray:
    from concourse.bass_utils import run_bass_kernel_spmd

    x, w21, V, ctot = _host_prep(inputs)
    B = x.shape[0]
    nc = build_program(ctot)
    in_maps = []
    for i in range(N_CORES):
        in_maps.append({
            "x": np.ascontiguousarray(x[i * S:(i + 1) * S]),
            "w21": w21,
            "v": V,
        })
    res = run_bass_kernel_spmd(nc, in_maps, list(range(N_CORES)))
    out = np.concatenate([res.results[i]["out"] for i in range(N_CORES)], axis=0)
    assert out.shape == (B, N, H)
    return out


# revision 11
# speedup vs baseline: 3.6393x; 1.0890x over previous
"""GAT-style message passing kernel for Trainium2 (8 NeuronCores, data-parallel over batch).

Reference math (per sample, 2 layers, shared weights):
    hidden = x @ W_in + b_in                      # [N, H]
    per layer:
        xt  = hidden @ W_t + b_t
        s_j = xt @ a_j ; s_i = xt @ a_i           # xt only feeds the scores
        att = softmax_j(lrelu(s_i[i] + s_j[j]))
        hidden = att @ hidden + hidden

Restructurings used here:
 1) W_t folding: s = hidden @ (W_t a) + b_t.a  — the NxHxH transform collapses.
 2) Rank-21 factorization: hidden == U @ V with V = [W_in; b_in] constant and
    U0 = [x | 1];  per layer U <- att @ U + U  (attention commutes with V).
    All attention matmuls run on U's 21 columns; V is applied once at the end.
    The ones-column of U doubles per layer (att rows sum to 1), and its output
    row in E.T @ U equals 2^L * D — the softmax denominator comes for free.
 3) exp(lrelu(z)-C_i) = max(e^{z-C_i}, e^{0.01z-C_i}) and with C_i = s_i+maxS
    both branches are rank-1:  E[j,i] = max(p[j], p'[j]*g[i])  with
    p = e^{s_j-maxS}, p' = e^{0.01(s_j-maxS)}, g = e^{min(-0.99(s_i+maxS+c),80)}
    so the whole N^2 pass is ONE fused DVE tensor_scalar per tile, no N^2 exp.
 4) s for the next layer from the same product: s' = rD * (Y_U @ w21) + s.
"""

import numpy as np
from contextlib import ExitStack

S = 2          # samples per core
N = 2048
Din = 20
UD = Din + 1   # U columns: 20 x-features + ones
H = 128
NCH = 16       # j-chunks of 128
NB = 4         # i-blocks
FB = 512       # i-block width
NUM_LAYERS = 2
N_CORES = 8


def _build(ctx, tc, aps, ctot):
    import concourse.bass as bass
    from concourse import mybir
    from concourse.masks import make_identity

    nc = tc.nc
    f32 = mybir.dt.float32
    f16 = mybir.dt.float16
    Alu = mybir.AluOpType
    Act = mybir.ActivationFunctionType

    x_ap, w21_ap, v_ap, out_ap = aps

    consts = ctx.enter_context(tc.tile_pool(name="consts", bufs=1))
    utp = ctx.enter_context(tc.tile_pool(name="utp", bufs=2))        # U0T / YUT rows [UD, N]
    natp = ctx.enter_context(tc.tile_pool(name="natp", bufs=4))      # U_nat f32 [128, 16, UD]
    natp16 = ctx.enter_context(tc.tile_pool(name="natp16", bufs=4))  # U_nat fp16
    ynat = ctx.enter_context(tc.tile_pool(name="ynat", bufs=2))      # Ynat f32 [128, 16, UD]
    xin = ctx.enter_context(tc.tile_pool(name="xin", bufs=4))        # x load tiles
    gpool = ctx.enter_context(tc.tile_pool(name="gpool", bufs=4))    # gbc [128, 512]
    epool = ctx.enter_context(tc.tile_pool(name="epool", bufs=8))    # E tiles [128, 512] f16
    outp = ctx.enter_context(tc.tile_pool(name="outp", bufs=2))      # final hidden [128,16,128]
    small = ctx.enter_context(tc.tile_pool(name="small", bufs=12))
    psA = ctx.enter_context(tc.tile_pool(name="psA", bufs=2, space="PSUM"))  # ubc [128,512]
    psU = ctx.enter_context(tc.tile_pool(name="psU", bufs=3, space="PSUM"))  # YUT [UD,512]
    psT = ctx.enter_context(tc.tile_pool(name="psT", bufs=3, space="PSUM"))  # transposes

    ident = consts.tile([128, 128], f32)
    make_identity(nc, ident)
    ones_r = consts.tile([1, 128], f32)
    nc.vector.memset(ones_r, 1.0)
    w21_sb = consts.tile([UD, 2], f32)
    nc.sync.dma_start(out=w21_sb, in_=w21_ap)
    v_sb = consts.tile([UD, H], f32)
    nc.sync.dma_start(out=v_sb, in_=v_ap)
    # selmat[k, c, p] = (c == k): broadcast row c of a [16,128] tile to all
    # 128 output partitions via one K=16 matmul.
    it1 = consts.tile([NCH, NCH, 128], mybir.dt.int32)
    nc.gpsimd.iota(it1, [[1, NCH], [0, 128]], channel_multiplier=0)
    it2 = consts.tile([NCH, NCH, 128], mybir.dt.int32)
    nc.gpsimd.iota(it2, [[0, NCH], [0, 128]], channel_multiplier=1)
    selmat = consts.tile([NCH, NCH, 128], f32)
    nc.vector.tensor_tensor(out=selmat, in0=it1, in1=it2, op=Alu.is_equal)

    def ts(out, in0, s1, s2, op0, op1=None):
        if op1 is None:
            nc.vector.tensor_scalar(out, in0, s1, None, op0)
        else:
            nc.vector.tensor_scalar(out, in0, s1, s2, op0, op1)

    # ------------- input stage: x -> U0 (natural + T), initial scores -------
    u_nat = [None, None]
    u_nat16 = [None, None]
    s_part = [None, None]   # biasless scores [128, 16, 2]
    for s in range(S):
        # one contiguous DMA; node n lives at (partition p, chunk c) with
        # n = 16 p + c — a fixed relabeling the attention sum is invariant to.
        xflat = xin.tile([128, NCH, Din], f32)
        nc.sync.dma_start(out=xflat, in_=x_ap[s].rearrange("(p c) d -> p c d", c=NCH))
        un = natp.tile([128, NCH, UD], f32, tag="unat")
        nc.vector.memset(un[:, :, Din:UD], 1.0)
        nc.vector.tensor_copy(un[:, :, 0:Din], xflat)
        u0t = utp.tile([UD, N], f32, tag="u0t")
        nc.vector.memset(u0t, 1.0)
        for c in range(NCH):
            pst = psT.tile([Din, 128], f32, tag="tp")
            nc.tensor.transpose(pst, xflat[:, c, :], ident)
            nc.scalar.copy(u0t[0:Din, c * 128:(c + 1) * 128], pst)
        un16 = natp16.tile([128, NCH, UD], f16, tag="unat16")
        nc.vector.tensor_copy(un16, un)
        # initial biasless scores s0[j, c, z] = U0[j] . w21[:, z]
        pss = psT.tile([128, 32], f32, tag="tp")
        for c in range(NCH):
            nc.tensor.matmul(pss[:, 2 * c:2 * c + 2], lhsT=u0t[:, c * 128:(c + 1) * 128],
                             rhs=w21_sb, start=True, stop=True)
        s0 = small.tile([128, NCH, 2], f32, tag="s0")
        nc.vector.tensor_copy(s0, pss.rearrange("p (c z) -> p c z", z=2))
        u_nat[s], u_nat16[s], s_part[s] = un, un16, s0

    # ------------- layers ---------------------------------------------------
    for L in range(NUM_LAYERS):
        last = L == NUM_LAYERS - 1
        for s in range(S):
            un, un16, s0 = u_nat[s], u_nat16[s], s_part[s]

            # global max of biasless s_j
            m1 = small.tile([128, 1], f32, tag="m1")
            nc.vector.tensor_reduce(m1, s0[:, :, 0], axis=mybir.AxisListType.X, op=Alu.max)
            psm = psT.tile([1, 128], f32, tag="tp")
            nc.tensor.matmul(psm, lhsT=m1, rhs=ident, start=True, stop=True)
            m1r = small.tile([1, 128], f32, tag="m1r")
            nc.scalar.copy(m1r, psm)
            mx = small.tile([1, 1], f32, tag="mx")
            nc.vector.tensor_reduce(mx, m1r, axis=mybir.AxisListType.X, op=Alu.max)
            psmb = psT.tile([128, 1], f32, tag="tp")
            nc.tensor.matmul(psmb, lhsT=ones_r, rhs=mx, start=True, stop=True)
            maxbc = small.tile([128, 1], f32, tag="maxbc")
            nc.vector.tensor_copy(maxbc, psmb)
            negmax = small.tile([128, 1], f32, tag="negmax")
            ts(negmax, maxbc, -1.0, None, Alu.mult)
            negmax001 = small.tile([128, 1], f32, tag="negmax001")
            ts(negmax001, maxbc, -0.01, None, Alu.mult)

            # p = exp(s_j - maxS), p' = exp(0.01(s_j - maxS))
            p_sb = small.tile([128, NCH], f32, tag="p_sb")
            nc.scalar.activation(p_sb, s0[:, :, 0], Act.Exp, bias=negmax[:, 0:1], scale=1.0)
            pp_sb = small.tile([128, NCH], f32, tag="pp_sb")
            nc.scalar.activation(pp_sb, s0[:, :, 0], Act.Exp, bias=negmax001[:, 0:1], scale=0.01)

            # u = min(-0.99(s_i + maxS + ctot), 80), then to row layout
            u1 = small.tile([128, NCH], f32, tag="u1")
            ts(u1, s0[:, :, 1], maxbc[:, 0:1], float(ctot), Alu.add, Alu.add)
            u_sb = small.tile([128, NCH], f32, tag="u_sb")
            ts(u_sb, u1, -0.99, 10.5, Alu.mult, Alu.min)
            psu = psT.tile([NCH, 128], f32, tag="tp")
            nc.tensor.transpose(psu, u_sb, ident)
            u_rows = small.tile([NCH, 128], f32, tag="u_rows")
            nc.scalar.copy(u_rows, psu)

            # attention sweep: Y_UT[u, i] = sum_j U[j, u] E[j, i]
            yut_sb = utp.tile([UD, N], f32, tag="yut")
            for b in range(NB):
                ubc = psA.tile([128, FB], f32, tag="ubc")
                for k in range(4):
                    c = 4 * b + k
                    nc.tensor.matmul(ubc[:, k * 128:(k + 1) * 128], lhsT=selmat[:, c, :],
                                     rhs=u_rows, start=True, stop=True)
                gbc = gpool.tile([128, FB], f16, tag="gbc")
                nc.scalar.activation(gbc, ubc, Act.Exp)

                yps = psU.tile([UD, FB], f32, tag="yps")
                for c in range(NCH):
                    e_t = epool.tile([128, FB], f16, tag="e")
                    ts(e_t, gbc, pp_sb[:, c:c + 1], p_sb[:, c:c + 1], Alu.mult, Alu.max)
                    nc.tensor.matmul(yps, lhsT=un16[:, c, :], rhs=e_t,
                                     start=(c == 0), stop=(c == NCH - 1))
                nc.scalar.copy(yut_sb[:, b * FB:(b + 1) * FB], yps)

            # transpose Y_UT to natural chunks; col Din carries 2^L * D
            yn = ynat.tile([128, NCH, UD], f32, tag="ynat")
            for c in range(NCH):
                pst = psT.tile([128, UD], f32, tag="tp")
                nc.tensor.transpose(pst, yut_sb[:, c * 128:(c + 1) * 128],
                                    ident[0:UD, 0:UD])
                nc.vector.tensor_copy(yn[:, c, :], pst)

            dsc = small.tile([128, NCH], f32, tag="dsc")
            ts(dsc, yn[:, :, Din], float(2.0 ** (-L)), None, Alu.mult)
            rd = small.tile([128, NCH], f32, tag="rd")
            nc.vector.reciprocal(rd, dsc)

            # U' = Ynat * rd + U  (also updates the ones-col to 2^{L+1})
            new_un = natp.tile([128, NCH, UD], f32, tag="unat")
            for c in range(NCH):
                nc.vector.scalar_tensor_tensor(new_un[:, c, :], yn[:, c, :],
                                               rd[:, c:c + 1], un[:, c, :],
                                               Alu.mult, Alu.add)

            if not last:
                new_un16 = natp16.tile([128, NCH, UD], f16, tag="unat16")
                nc.vector.tensor_copy(new_un16, new_un)
                # next-layer biasless scores: s' = rd * (Y_UT @ w21) + s
                psq = psT.tile([128, 32], f32, tag="tp")
                for c in range(NCH):
                    nc.tensor.matmul(psq[:, 2 * c:2 * c + 2],
                                     lhsT=yut_sb[:, c * 128:(c + 1) * 128],
                                     rhs=w21_sb, start=True, stop=True)
                qp = small.tile([128, NCH, 2], f32, tag="qp")
                nc.vector.tensor_copy(qp, psq.rearrange("p (c z) -> p c z", z=2))
                new_s0 = small.tile([128, NCH, 2], f32, tag="s0")
                for c in range(NCH):
                    nc.vector.scalar_tensor_tensor(new_s0[:, c, :], qp[:, c, :],
                                                   rd[:, c:c + 1], s0[:, c, :],
                                                   Alu.mult, Alu.add)
                u_nat[s], u_nat16[s], s_part[s] = new_un, new_un16, new_s0
            else:
                # hidden = U' @ V, emitted per chunk
                hout = outp.tile([128, NCH, H], f32, tag="hout")
                for c in range(NCH):
                    psut = psT.tile([UD, 128], f32, tag="tp")
                    nc.tensor.transpose(psut, new_un[:, c, :], ident)
                    u2t_c = small.tile([UD, 128], f32, tag="u2t")
                    nc.scalar.copy(u2t_c, psut)
                    psh = psT.tile([128, H], f32, tag="tp")
                    nc.tensor.matmul(psh, lhsT=u2t_c, rhs=v_sb, start=True, stop=True)
                    nc.scalar.copy(hout[:, c, :], psh)
                nc.sync.dma_start(
                    out=out_ap[s].rearrange("(p c) h -> p c h", c=NCH),
                    in_=hout)


def _host_prep(inputs):
    x = np.ascontiguousarray(np.asarray(inputs["x"], dtype=np.float32))
    W_in = np.asarray(inputs["W_in"], dtype=np.float32)
    b_in = np.asarray(inputs["b_in"], dtype=np.float32)
    W_t = np.asarray(inputs["W_t"], dtype=np.float32)
    b_t = np.asarray(inputs["b_t"], dtype=np.float32)
    a = np.asarray(inputs["a"], dtype=np.float32)
    a_j, a_i = a[:H, 0], a[H:, 0]
    wj = (W_t @ a_j).astype(np.float32)
    wi = (W_t @ a_i).astype(np.float32)
    V = np.ascontiguousarray(np.concatenate([W_in, b_in[None, :]], axis=0))  # [21, 128]
    w21 = np.ascontiguousarray(np.stack([V @ wj, V @ wi], axis=1))           # [21, 2]
    ctot = float(np.float32(b_t @ a_j) + np.float32(b_t @ a_i))
    return x, w21, V, ctot


def build_program(ctot):
    import concourse.tile as tile
    from concourse import mybir
    from concourse.bacc import Bacc

    f32 = mybir.dt.float32
    nc = Bacc("TRN2", target_bir_lowering=False, debug=False)
    x_t = nc.dram_tensor("x", [S, N, Din], f32, kind="ExternalInput")
    w21_t = nc.dram_tensor("w21", [UD, 2], f32, kind="ExternalInput")
    v_t = nc.dram_tensor("v", [UD, H], f32, kind="ExternalInput")
    out_t = nc.dram_tensor("out", [S, N, H], f32, kind="ExternalOutput")
    aps = (x_t.ap(), w21_t.ap(), v_t.ap(), out_t.ap())
    with tile.TileContext(nc) as tc, ExitStack() as ctx:
        _build(ctx, tc, aps, ctot)
    nc.compile()
    return nc


def kernel(**inputs) -> np.ndarray:
    from concourse.bass_utils import run_bass_kernel_spmd

    x, w21, V, ctot = _host_prep(inputs)
    B = x.shape[0]
    nc = build_program(ctot)
    in_maps = []
    for i in range(N_CORES):
        in_maps.append({
            "x": np.ascontiguousarray(x[i * S:(i + 1) * S]),
            "w21": w21,
            "v": V,
        })
    res = run_bass_kernel_spmd(nc, in_maps, list(range(N_CORES)))
    out = np.concatenate([res.results[i]["out"] for i in range(N_CORES)], axis=0)
    assert out.shape == (B, N, H)
    return out
